# revision 10
# baseline (speedup 1.0000x reference)
"""Trainium2 Bass kernel for an 8-expert MoE FFN layer (nn_MoELayer).

Reference computation (per expert e over its contiguous 1024-token chunk):
    h = gelu(x_e @ w1[e] + b1[e]);  y_e = h @ w2[e] + b2[e]

Sharding: expert parallelism — core e holds expert e's weights and its token
chunk (the gate yields equal contiguous chunks, so no all-to-all is needed).
Each core runs the same SPMD program on its own data.

Per-core kernel (T=1024 tokens, D=1024, F=4096), all matmuls in fp16 with
fp32 PSUM accumulation (215.5 ns per 512-wide matmul incl. fast weight load —
the PE's measured floor; fp32 would be 4x slower):
  phase 1: for each 128-wide f-tile: h^T[ft] = gelu(w1[:,ft]^T @ x^T + b1[ft])
           (f on partitions -> b1 is a per-partition ACT bias; h^T resident in SBUF)
  phase 2: for each 128-wide dm-tile: y^T[dmo] = w2[:,dmo]^T @ h^T + b2[dmo]
           chunk-major (512-col PSUM groups) so y flushes stream in small
           pieces throughout phase 2 instead of per-dmo bursts at the end
All layout transposes/repacks are done on the host so every DMA is a large
partition-contiguous stream. Head DMAs are split across the Sync (w1/biases)
and Scalar (x) queues so descriptor generation is not serialized; x chunk 0
streams in do-tile-sized pieces so the first matmul group starts ~2.3us after
the engines clear the entry barrier. A short burst of dummy matmuls on
scratch data warms the PE clock (HAM) while the first pieces land.
"""

import os

import numpy as np

# The kernel executes through the axon PJRT backend; a CPU pin (e.g. set for
# a jax reference run) would break NEFF dispatch in this process.
if os.environ.get("JAX_PLATFORMS") == "cpu":
    del os.environ["JAX_PLATFORMS"]

E = 8          # experts == cores
B, S = 2, 4096
D = 1024       # d_model
F = 4096       # d_ff
T = (B * S) // E  # tokens per expert chunk = 1024
P = 128
DO = D // P    # 8  k-tiles of d_model
FT = F // P    # 32 f-tiles of d_ff
DMO = D // P   # 8  output dm-tiles
FT2 = FT // 2  # half-slab of w2 f-tiles
NCHUNK = T // 512  # 2 moving-operand chunks (PSUM bank caps matmul N at 512)
N_WARMUP_MM = 16
HEAD = 4       # f-tiles that run chunk-0 first while x chunk 1 streams

_cached = None


def _build():
    import concourse.mybir as mybir
    import concourse.tile as tile
    from concourse import bacc
    from concourse.tile_rust import add_dep_helper

    f32 = mybir.dt.float32
    f16 = mybir.dt.float16

    nc = bacc.Bacc("TRN2", target_bir_lowering=False, debug=False, num_devices=E)

    xT_d = nc.dram_tensor("xT", [NCHUNK, P, DO, 512], f16, kind="ExternalInput")
    w1_d = nc.dram_tensor("w1r", [FT, P, DO, P], f16, kind="ExternalInput")
    bc_d = nc.dram_tensor("bc", [P, FT + DMO], f32, kind="ExternalInput")
    w2_d = nc.dram_tensor("w2r", [DMO, 2, P, FT2, P], f16, kind="ExternalInput")
    yT_d = nc.dram_tensor("yT", [DMO, P, T], f32, kind="ExternalOutput")

    gelu = mybir.ActivationFunctionType.Gelu_apprx_tanh

    with tile.TileContext(nc) as tc:
        with (
            tc.tile_pool(name="xpool", bufs=1) as xpool,
            tc.tile_pool(name="hpool", bufs=1) as hpool,
            tc.tile_pool(name="wpool", bufs=2) as wpool,
            tc.tile_pool(name="cpool", bufs=1) as cpool,
            tc.tile_pool(name="ypool", bufs=2) as ypool,
            tc.tile_pool(name="psum_h", bufs=2, space="PSUM") as psum_h,
            tc.tile_pool(name="psum_y", bufs=2, space="PSUM") as psum_y,
        ):
            # scratch for PE warmup, prepared before anything else queues
            scratch = cpool.tile([P, 512], f16)
            nc.gpsimd.memset(scratch[:], 0.0)

            # Head DMAs. The sync and scalar engines each feed their own
            # hardware DGE ring (FIFO per ring; the 16 SDMA engines
            # round-robin between rings), so: the critical-path stream (x
            # chunk 0 + early w1 tiles, in exact consumption order) goes on
            # the sync ring at full rate, while the slower scalar ring
            # carries a few late-need w1 tiles to add aggregate bandwidth.
            w1_tiles = {}
            for ft in range(8):
                w1_tiles[ft] = wpool.tile(
                    [P, DO, P], f16, tag="w1", bufs=8, name="w1_sb"
                )
            # contiguous-per-partition destination: [p, c, do*512]
            xT_sb = xpool.tile([P, NCHUNK, DO * 512], f16)
            xc0 = xT_d.ap()[0].rearrange("p do t -> p (do t)")
            xc1 = xT_d.ap()[1].rearrange("p do t -> p (do t)")
            W = 512  # one do-tile worth of columns
            HX = DO * 512 // 2

            nc.sync.dma_start(w1_tiles[0][:], w1_d.ap()[0])
            bc_sb = cpool.tile([P, FT + DMO], f32)
            nc.sync.dma_start(bc_sb[:], bc_d.ap())
            nc.sync.dma_start(xT_sb[:, 0, : 4 * W], xc0[:, : 4 * W])
            nc.sync.dma_start(xT_sb[:, 0, 4 * W : 6 * W], xc0[:, 4 * W : 6 * W])
            nc.sync.dma_start(xT_sb[:, 0, 6 * W :], xc0[:, 6 * W :])
            nc.sync.dma_start(w1_tiles[1][:], w1_d.ap()[1])
            nc.sync.dma_start(w1_tiles[2][:], w1_d.ap()[2])
            nc.sync.dma_start(xT_sb[:, 1, :HX], xc1[:, :HX])
            nc.sync.dma_start(xT_sb[:, 1, HX:], xc1[:, HX:])
            nc.sync.dma_start(w1_tiles[3][:], w1_d.ap()[3])
            nc.sync.dma_start(w1_tiles[4][:], w1_d.ap()[4])
            nc.sync.dma_start(w1_tiles[5][:], w1_d.ap()[5])
            nc.sync.dma_start(w1_tiles[6][:], w1_d.ap()[6])
            nc.sync.dma_start(w1_tiles[7][:], w1_d.ap()[7])
            b1_sb = bc_sb[:, :FT]
            b2_sb = bc_sb[:, FT:]

            # PE warmup: dummy matmuls on scratch while the first DMAs land.
            # Keeps the HAM clock-gate fed until real work arrives.
            for i in range(N_WARMUP_MM):
                pw = psum_y.tile([P, 512], f32, tag="py", bufs=4, name="pwarm")
                nc.tensor.matmul(
                    pw[:], scratch[:, :P], scratch[:], start=True, stop=True
                )

            h_sb = hpool.tile([P, FT, T], f16)

            # ---- phase 1: h^T = gelu(w1^T x^T + b1), one 128-row f-tile at a time
            # per-(ft, chunk) 1-bank PSUM tiles; the first HEAD f-tiles run
            # chunk-0 first so the PE streams while x chunk 1 is on the wire
            def mm1_group(ph, w1_sb, c):
                mm = None
                for do in range(DO):
                    mm = nc.tensor.matmul(
                        ph[:],
                        w1_sb[:, do, :],
                        xT_sb[:, c, do * 512 : (do + 1) * 512],
                        start=(do == 0),
                        stop=(do == DO - 1),
                    )
                return mm

            def gelu_chunk(ph, ft, c):
                cs = slice(c * 512, (c + 1) * 512)
                return nc.scalar.activation(
                    h_sb[:, ft, cs], ph[:], gelu, bias=b1_sb[:, ft : ft + 1]
                )

            gelu_insts = {}
            head_ph = {}
            for ft in range(HEAD):
                ph = psum_h.tile([P, 512], f32, tag="ph", bufs=4, name="ph")
                head_ph[ft] = ph
                mm1_group(ph, w1_tiles[ft], 0)
            for ft in range(HEAD):
                ph = head_ph[ft]
                gelu_insts[(ft, 0)] = gelu_chunk(ph, ft, 0)
                ph2 = psum_h.tile([P, 512], f32, tag="ph", bufs=4, name="ph")
                mm1_group(ph2, w1_tiles[ft], 1)
                gelu_insts[(ft, 1)] = gelu_chunk(ph2, ft, 1)

            for ft in range(HEAD, FT):
                if ft not in w1_tiles:
                    w1_tiles[ft] = wpool.tile(
                        [P, DO, P], f16, tag="w1", bufs=8, name="w1_sb"
                    )
                    nc.sync.dma_start(w1_tiles[ft][:], w1_d.ap()[ft])
                w1_sb = w1_tiles[ft]
                for c in range(NCHUNK):
                    ph = psum_h.tile([P, 512], f32, tag="ph", bufs=4, name="ph")
                    mm1_group(ph, w1_sb, c)
                    gelu_insts[(ft, c)] = gelu_chunk(ph, ft, c)

            # ---- phase 2: y^T[dmo] = w2[:,dmo]^T h^T + b2[dmo], chunk-major:
            # each (dmo, c) group accumulates a 1-bank 512-col PSUM tile and
            # flushes it in 256-col pieces so the y DMA stream never bursts.
            # The very last group runs as two 256-col PSUM tiles so half its
            # flush overlaps the final matmuls, and its last DMA issues from
            # the (idle) scalar queue instead of the congested sync queue.
            FQ = FT // 4
            for dmo in range(DMO):
                w2_q = []
                for qq in range(4):
                    w2_sb = wpool.tile([P, FQ, P], f16, tag="w2", bufs=32, name="w2_sb")
                    dma = nc.sync.dma_start(
                        w2_sb[:],
                        w2_d.ap()[dmo, qq // 2, :, (qq % 2) * FQ : (qq % 2 + 1) * FQ],
                    )
                    if dmo == 0:
                        # keep dmo 0's prefetch out of the head's w1/xT window
                        add_dep_helper(
                            dma.ins,
                            gelu_insts[(6, 1)].ins,
                            sync=True,
                            reason="delay w2 prefetch past the kernel head",
                        )
                    w2_q.append(w2_sb)

                last_dmo = dmo == DMO - 1
                for c in range(NCHUNK):
                    last_group = last_dmo and c == NCHUNK - 1
                    if not last_group:
                        cs = slice(c * 512, (c + 1) * 512)
                        py = psum_y.tile([P, 512], f32, tag="py", bufs=4, name="py")
                        for fo in range(FT):
                            wt = w2_q[fo // FQ][:, fo % FQ, :]
                            nc.tensor.matmul(
                                py[:],
                                wt,
                                h_sb[:, fo, cs],
                                start=(fo == 0),
                                stop=(fo == FT - 1),
                            )
                        for cc in range(2):
                            ccs = slice(c * 512 + cc * 256, c * 512 + (cc + 1) * 256)
                            y_sb = ypool.tile([P, 256], f32, tag="y", bufs=4, name="y_sb")
                            nc.vector.tensor_scalar_add(
                                y_sb[:],
                                py[:, cc * 256 : (cc + 1) * 256],
                                b2_sb[:, dmo : dmo + 1],
                            )
                            # late y flushes ride the (idle) scalar ring so
                            # the sync ring's tail FIFO stays short
                            eng = nc.scalar if dmo >= 5 else nc.sync
                            eng.dma_start(yT_d.ap()[dmo, :, ccs], y_sb[:])
                    else:
                        for half in range(2):
                            hs = slice(c * 512 + half * 256, c * 512 + (half + 1) * 256)
                            py = psum_y.tile([P, 256], f32, tag="py", bufs=4, name="pyl")
                            for fo in range(FT):
                                wt = w2_q[fo // FQ][:, fo % FQ, :]
                                nc.tensor.matmul(
                                    py[:],
                                    wt,
                                    h_sb[:, fo, hs],
                                    start=(fo == 0),
                                    stop=(fo == FT - 1),
                                )
                            y_sb = ypool.tile([P, 256], f32, tag="y", bufs=4, name="y_sb")
                            nc.vector.tensor_scalar_add(
                                y_sb[:], py[:], b2_sb[:, dmo : dmo + 1]
                            )
                            eng = nc.scalar if half == 1 else nc.sync
                            eng.dma_start(yT_d.ap()[dmo, :, hs], y_sb[:])

    nc.compile()
    return nc


def _get_nc():
    global _cached
    if _cached is None:
        _cached = _build()
    return _cached


def make_in_maps(x, w1, b1, w2, b2):
    x = np.asarray(x, dtype=np.float32)
    w1 = np.asarray(w1, dtype=np.float32)
    b1 = np.asarray(b1, dtype=np.float32)
    w2 = np.asarray(w2, dtype=np.float32)
    b2 = np.asarray(b2, dtype=np.float32)

    tokens = x.reshape(E, T, D)
    in_maps = []
    for e in range(E):
        xT = np.ascontiguousarray(
            tokens[e].reshape(NCHUNK, 512, DO, P).transpose(0, 3, 2, 1)
        ).astype(np.float16)  # [c, p, do, t']
        w1r = np.ascontiguousarray(
            w1[e].reshape(DO, P, FT, P).transpose(2, 1, 0, 3)
        ).astype(np.float16)  # [ft, p, do, j]
        bc = np.ascontiguousarray(
            np.concatenate([b1[e].reshape(FT, P).T, b2[e].reshape(DMO, P).T], axis=1)
        )  # [p, ft..dmo]
        w2r = np.ascontiguousarray(
            w2[e].reshape(2, FT2, P, DMO, P).transpose(3, 0, 2, 1, 4)
        ).astype(np.float16)  # [dmo, half, p, fo, j]
        in_maps.append({"xT": xT, "w1r": w1r, "bc": bc, "w2r": w2r})
    return in_maps


def gather_out(results):
    out = np.empty((E, T, D), dtype=np.float32)
    for e in range(E):
        yT = results[e]["yT"]  # [dmo, p, t]
        out[e] = yT.transpose(2, 0, 1).reshape(T, D)
    return out.reshape(B, S, D)


def kernel(x, w1, b1, w2, b2):
    from concourse.bass_utils import run_bass_kernel_spmd

    nc = _get_nc()
    in_maps = make_in_maps(x, w1, b1, w2, b2)
    res = run_bass_kernel_spmd(nc, in_maps, core_ids=list(range(E)))
    return gather_out(res.results)


# revision 18
# speedup vs baseline: 1.0002x; 1.0002x over previous
"""Trainium2 Bass kernel for an 8-expert MoE FFN layer (nn_MoELayer).

Reference computation (per expert e over its contiguous 1024-token chunk):
    h = gelu(x_e @ w1[e] + b1[e]);  y_e = h @ w2[e] + b2[e]

Sharding: expert parallelism — core e holds expert e's weights and its token
chunk (the gate yields equal contiguous chunks, so no all-to-all is needed).
Each core runs the same SPMD program on its own data.

Per-core kernel (T=1024 tokens, D=1024, F=4096), all matmuls in fp16 with
fp32 PSUM accumulation (215.5 ns per 512-wide matmul incl. fast weight load —
the PE's measured floor; fp32 would be 4x slower):
  phase 1: for each 128-wide f-tile: h^T[ft] = gelu(w1[:,ft]^T @ x^T + b1[ft])
           (f on partitions -> b1 is a per-partition ACT bias; h^T resident in SBUF)
  phase 2: for each 128-wide dm-tile: y^T[dmo] = w2[:,dmo]^T @ h^T + b2[dmo]
           chunk-major (512-col PSUM groups) so y flushes stream in small
           pieces throughout phase 2 instead of per-dmo bursts at the end
All layout transposes/repacks are done on the host so every DMA is a large
partition-contiguous stream. Head DMAs are split across the Sync (w1/biases)
and Scalar (x) queues so descriptor generation is not serialized; x chunk 0
streams in do-tile-sized pieces so the first matmul group starts ~2.3us after
the engines clear the entry barrier. A short burst of dummy matmuls on
scratch data warms the PE clock (HAM) while the first pieces land.
"""

import os

import numpy as np

# The kernel executes through the axon PJRT backend; a CPU pin (e.g. set for
# a jax reference run) would break NEFF dispatch in this process.
if os.environ.get("JAX_PLATFORMS") == "cpu":
    del os.environ["JAX_PLATFORMS"]

E = 8          # experts == cores
B, S = 2, 4096
D = 1024       # d_model
F = 4096       # d_ff
T = (B * S) // E  # tokens per expert chunk = 1024
P = 128
DO = D // P    # 8  k-tiles of d_model
FT = F // P    # 32 f-tiles of d_ff
DMO = D // P   # 8  output dm-tiles
FT2 = FT // 2  # half-slab of w2 f-tiles
NCHUNK = T // 512  # 2 moving-operand chunks (PSUM bank caps matmul N at 512)
N_WARMUP_MM = 16
HEAD = 4       # f-tiles that run chunk-0 first while x chunk 1 streams

_cached = None


def _build():
    import concourse.mybir as mybir
    import concourse.tile as tile
    from concourse import bacc
    from concourse.tile_rust import add_dep_helper

    f32 = mybir.dt.float32
    f16 = mybir.dt.float16
    bf16 = mybir.dt.bfloat16

    nc = bacc.Bacc("TRN2", target_bir_lowering=False, debug=False, num_devices=E)

    xT_d = nc.dram_tensor("xT", [NCHUNK, P, DO, 512], f16, kind="ExternalInput")
    w1_d = nc.dram_tensor("w1r", [FT, P, DO, P], f16, kind="ExternalInput")
    bc_d = nc.dram_tensor("bc", [P, FT + DMO], f32, kind="ExternalInput")
    w2_d = nc.dram_tensor("w2r", [DMO, 2, P, FT2, P], f16, kind="ExternalInput")
    yT_d = nc.dram_tensor("yT", [DMO, P, T], f32, kind="ExternalOutput")

    gelu = mybir.ActivationFunctionType.Gelu_apprx_tanh

    with tile.TileContext(nc) as tc:
        with (
            tc.tile_pool(name="xpool", bufs=1) as xpool,
            tc.tile_pool(name="hpool", bufs=1) as hpool,
            tc.tile_pool(name="wpool", bufs=2) as wpool,
            tc.tile_pool(name="cpool", bufs=1) as cpool,
            tc.tile_pool(name="ypool", bufs=2) as ypool,
            tc.tile_pool(name="psum_h", bufs=2, space="PSUM") as psum_h,
            tc.tile_pool(name="psum_y", bufs=2, space="PSUM") as psum_y,
        ):
            # scratch for PE warmup, prepared before anything else queues
            scratch = cpool.tile([P, 512], f16)
            nc.gpsimd.memset(scratch[:], 0.0)

            # Head DMAs. The sync and scalar engines each feed their own
            # hardware DGE ring (FIFO per ring; the 16 SDMA engines
            # round-robin between rings), so: the critical-path stream (x
            # chunk 0 + early w1 tiles, in exact consumption order) goes on
            # the sync ring at full rate, while the slower scalar ring
            # carries a few late-need w1 tiles to add aggregate bandwidth.
            w1_tiles = {}
            for ft in range(8):
                w1_tiles[ft] = wpool.tile(
                    [P, DO, P], f16, tag="w1", bufs=8, name="w1_sb"
                )
            # contiguous-per-partition destination: [p, c, do*512]
            xT_sb = xpool.tile([P, NCHUNK, DO * 512], f16)
            xc0 = xT_d.ap()[0].rearrange("p do t -> p (do t)")
            xc1 = xT_d.ap()[1].rearrange("p do t -> p (do t)")
            W = 512  # one do-tile worth of columns
            HX = DO * 512 // 2

            nc.sync.dma_start(w1_tiles[0][:], w1_d.ap()[0])
            bc_sb = cpool.tile([P, FT + DMO], f32)
            nc.sync.dma_start(bc_sb[:], bc_d.ap())
            nc.sync.dma_start(xT_sb[:, 0, : 4 * W], xc0[:, : 4 * W])
            nc.sync.dma_start(xT_sb[:, 0, 4 * W : 6 * W], xc0[:, 4 * W : 6 * W])
            nc.sync.dma_start(xT_sb[:, 0, 6 * W :], xc0[:, 6 * W :])
            nc.sync.dma_start(w1_tiles[1][:], w1_d.ap()[1])
            nc.sync.dma_start(w1_tiles[2][:], w1_d.ap()[2])
            nc.sync.dma_start(xT_sb[:, 1, :HX], xc1[:, :HX])
            nc.sync.dma_start(xT_sb[:, 1, HX:], xc1[:, HX:])
            nc.sync.dma_start(w1_tiles[3][:], w1_d.ap()[3])
            nc.sync.dma_start(w1_tiles[4][:], w1_d.ap()[4])
            nc.sync.dma_start(w1_tiles[5][:], w1_d.ap()[5])
            nc.sync.dma_start(w1_tiles[6][:], w1_d.ap()[6])
            nc.sync.dma_start(w1_tiles[7][:], w1_d.ap()[7])
            b1_sb = bc_sb[:, :FT]
            b2_sb = bc_sb[:, FT:]

            # PE warmup: dummy matmuls on scratch while the first DMAs land.
            # Keeps the HAM clock-gate fed until real work arrives.
            for i in range(N_WARMUP_MM):
                pw = psum_y.tile([P, 512], f32, tag="py", bufs=4, name="pwarm")
                nc.tensor.matmul(
                    pw[:], scratch[:, :P], scratch[:], start=True, stop=True
                )

            h_sb = hpool.tile([P, FT, T], f16)

            # ---- phase 1: h^T = gelu(w1^T x^T + b1), one 128-row f-tile at a time
            # per-(ft, chunk) 1-bank PSUM tiles; the first HEAD f-tiles run
            # chunk-0 first so the PE streams while x chunk 1 is on the wire
            def mm1_group(ph, w1_sb, c):
                mm = None
                for do in range(DO):
                    mm = nc.tensor.matmul(
                        ph[:],
                        w1_sb[:, do, :],
                        xT_sb[:, c, do * 512 : (do + 1) * 512],
                        start=(do == 0),
                        stop=(do == DO - 1),
                    )
                return mm

            def gelu_chunk(ph, ft, c):
                cs = slice(c * 512, (c + 1) * 512)
                return nc.scalar.activation(
                    h_sb[:, ft, cs], ph[:], gelu, bias=b1_sb[:, ft : ft + 1]
                )

            gelu_insts = {}
            head_ph = {}
            for ft in range(HEAD):
                ph = psum_h.tile([P, 512], f32, tag="ph", bufs=4, name="ph")
                head_ph[ft] = ph
                mm1_group(ph, w1_tiles[ft], 0)
            for ft in range(HEAD):
                ph = head_ph[ft]
                gelu_insts[(ft, 0)] = gelu_chunk(ph, ft, 0)
                ph2 = psum_h.tile([P, 512], f32, tag="ph", bufs=4, name="ph")
                mm1_group(ph2, w1_tiles[ft], 1)
                gelu_insts[(ft, 1)] = gelu_chunk(ph2, ft, 1)

            for ft in range(HEAD, FT):
                if ft not in w1_tiles:
                    w1_tiles[ft] = wpool.tile(
                        [P, DO, P], f16, tag="w1", bufs=8, name="w1_sb"
                    )
                    nc.sync.dma_start(w1_tiles[ft][:], w1_d.ap()[ft])
                w1_sb = w1_tiles[ft]
                for c in range(NCHUNK):
                    ph = psum_h.tile([P, 512], f32, tag="ph", bufs=4, name="ph")
                    mm1_group(ph, w1_sb, c)
                    gelu_insts[(ft, c)] = gelu_chunk(ph, ft, c)

            # ---- phase 2: y^T[dmo] = w2[:,dmo]^T h^T + b2[dmo], chunk-major:
            # each (dmo, c) group accumulates a 1-bank 512-col PSUM tile and
            # flushes it in 256-col pieces so the y DMA stream never bursts.
            # The very last group runs as two 256-col PSUM tiles so half its
            # flush overlaps the final matmuls, and its last DMA issues from
            # the (idle) scalar queue instead of the congested sync queue.
            FQ = FT // 4
            for dmo in range(DMO):
                w2_q = []
                for qq in range(4):
                    w2_sb = wpool.tile([P, FQ, P], f16, tag="w2", bufs=32, name="w2_sb")
                    dma = nc.sync.dma_start(
                        w2_sb[:],
                        w2_d.ap()[dmo, qq // 2, :, (qq % 2) * FQ : (qq % 2 + 1) * FQ],
                    )
                    if dmo == 0:
                        # keep dmo 0's prefetch out of the head's w1/xT window
                        add_dep_helper(
                            dma.ins,
                            gelu_insts[(6, 1)].ins,
                            sync=True,
                            reason="delay w2 prefetch past the kernel head",
                        )
                    w2_q.append(w2_sb)

                if dmo < DMO - 1:
                    for c in range(NCHUNK):
                        cs = slice(c * 512, (c + 1) * 512)
                        py = psum_y.tile([P, 512], f32, tag="py", bufs=4, name="py")
                        for fo in range(FT):
                            wt = w2_q[fo // FQ][:, fo % FQ, :]
                            nc.tensor.matmul(
                                py[:],
                                wt,
                                h_sb[:, fo, cs],
                                start=(fo == 0),
                                stop=(fo == FT - 1),
                            )
                        for cc in range(2):
                            ccs = slice(c * 512 + cc * 256, c * 512 + (cc + 1) * 256)
                            y_sb = ypool.tile([P, 256], f32, tag="y", bufs=4, name="y_sb")
                            nc.vector.tensor_scalar_add(
                                y_sb[:],
                                py[:, cc * 256 : (cc + 1) * 256],
                                b2_sb[:, dmo : dmo + 1],
                            )
                            # late y flushes ride the (idle) scalar ring so
                            # the sync ring's tail FIFO stays short
                            eng = nc.scalar if dmo >= 5 else nc.sync
                            eng.dma_start(yT_d.ap()[dmo, :, ccs], y_sb[:])
                else:
                    # last dm-tile, chunk-major in progressively smaller
                    # PSUM groups (512, 256, 128, 128 cols) so only a 128-col
                    # add + 64KB DMA trail the final matmul
                    for lo, w in ((0, 512), (512, 256), (768, 128), (896, 128)):
                        gs = slice(lo, lo + w)
                        py = psum_y.tile([P, w], f32, tag="py", bufs=4, name="pyl")
                        for fo in range(FT):
                            wt = w2_q[fo // FQ][:, fo % FQ, :]
                            nc.tensor.matmul(
                                py[:],
                                wt,
                                h_sb[:, fo, gs],
                                start=(fo == 0),
                                stop=(fo == FT - 1),
                            )
                        nf = 2 if w == 512 else 1
                        for cc in range(nf):
                            fw = w // nf
                            ccs = slice(lo + cc * fw, lo + (cc + 1) * fw)
                            y_sb = ypool.tile([P, fw], f32, tag="y", bufs=4, name="y_sb")
                            nc.vector.tensor_scalar_add(
                                y_sb[:],
                                py[:, cc * fw : (cc + 1) * fw],
                                b2_sb[:, dmo : dmo + 1],
                            )
                            nc.scalar.dma_start(yT_d.ap()[dmo, :, ccs], y_sb[:])

    nc.compile()
    return nc


def _get_nc():
    global _cached
    if _cached is None:
        _cached = _build()
    return _cached


def make_in_maps(x, w1, b1, w2, b2):
    x = np.asarray(x, dtype=np.float32)
    w1 = np.asarray(w1, dtype=np.float32)
    b1 = np.asarray(b1, dtype=np.float32)
    w2 = np.asarray(w2, dtype=np.float32)
    b2 = np.asarray(b2, dtype=np.float32)

    tokens = x.reshape(E, T, D)
    in_maps = []
    for e in range(E):
        xT = np.ascontiguousarray(
            tokens[e].reshape(NCHUNK, 512, DO, P).transpose(0, 3, 2, 1)
        ).astype(np.float16)  # [c, p, do, t']
        w1r = np.ascontiguousarray(
            w1[e].reshape(DO, P, FT, P).transpose(2, 1, 0, 3)
        ).astype(np.float16)  # [ft, p, do, j]
        bc = np.ascontiguousarray(
            np.concatenate([b1[e].reshape(FT, P).T, b2[e].reshape(DMO, P).T], axis=1)
        )  # [p, ft..dmo]
        w2r = np.ascontiguousarray(
            w2[e].reshape(2, FT2, P, DMO, P).transpose(3, 0, 2, 1, 4)
        ).astype(np.float16)  # [dmo, half, p, fo, j]
        in_maps.append({"xT": xT, "w1r": w1r, "bc": bc, "w2r": w2r})
    return in_maps


def gather_out(results, b2=None):
    out = np.empty((E, T, D), dtype=np.float32)
    for e in range(E):
        yT = results[e]["yT"]  # [dmo, p, t]
        out[e] = yT.transpose(2, 0, 1).reshape(T, D)
    return out.reshape(B, S, D)


def kernel(x, w1, b1, w2, b2):
    from concourse.bass_utils import run_bass_kernel_spmd

    nc = _get_nc()
    in_maps = make_in_maps(x, w1, b1, w2, b2)
    res = run_bass_kernel_spmd(nc, in_maps, core_ids=list(range(E)))
    return gather_out(res.results, b2)


# revision 23
# speedup vs baseline: 1.0147x; 1.0144x over previous
"""Trainium2 Bass kernel for an 8-expert MoE FFN layer (nn_MoELayer).

Reference computation (per expert e over its contiguous 1024-token chunk):
    h = gelu(x_e @ w1[e] + b1[e]);  y_e = h @ w2[e] + b2[e]

Sharding: expert parallelism — core e holds expert e's weights and its token
chunk (the gate yields equal contiguous chunks, so no all-to-all is needed).
Each core runs the same SPMD program on its own data.

Per-core kernel (T=1024 tokens, D=1024, F=4096), all matmuls in fp16 with
fp32 PSUM accumulation (215.5 ns per 512-wide matmul incl. fast weight load —
the PE's measured floor; fp32 would be 4x slower):
  phase 1: for each 128-wide f-tile: h^T[ft] = gelu(w1[:,ft]^T @ x^T + b1[ft])
           (f on partitions -> b1 is a per-partition ACT bias; h^T resident in SBUF)
  phase 2: for each 128-wide dm-tile: y^T[dmo] = w2[:,dmo]^T @ h^T + b2[dmo]
           chunk-major (512-col PSUM groups) so y flushes stream in small
           pieces throughout phase 2 instead of per-dmo bursts at the end
All layout transposes/repacks are done on the host so every DMA is a large
partition-contiguous stream. Head DMAs are split across the Sync (w1/biases)
and Scalar (x) queues so descriptor generation is not serialized; x chunk 0
streams in do-tile-sized pieces so the first matmul group starts ~2.3us after
the engines clear the entry barrier. A short burst of dummy matmuls on
scratch data warms the PE clock (HAM) while the first pieces land.
"""

import os

import numpy as np

# The kernel executes through the axon PJRT backend; a CPU pin (e.g. set for
# a jax reference run) would break NEFF dispatch in this process.
if os.environ.get("JAX_PLATFORMS") == "cpu":
    del os.environ["JAX_PLATFORMS"]

E = 8          # experts == cores
B, S = 2, 4096
D = 1024       # d_model
F = 4096       # d_ff
T = (B * S) // E  # tokens per expert chunk = 1024
P = 128
DO = D // P    # 8  k-tiles of d_model
FT = F // P    # 32 f-tiles of d_ff
DMO = D // P   # 8  output dm-tiles
FT2 = FT // 2  # half-slab of w2 f-tiles
NCHUNK = T // 512  # 2 moving-operand chunks (PSUM bank caps matmul N at 512)
N_WARMUP_MM = 12

_cached = None


def _build():
    import concourse.mybir as mybir
    import concourse.tile as tile
    from concourse import bacc
    from concourse.tile_rust import add_dep_helper

    f32 = mybir.dt.float32
    f16 = mybir.dt.float16
    bf16 = mybir.dt.bfloat16

    nc = bacc.Bacc("TRN2", target_bir_lowering=False, debug=False, num_devices=E)

    xT_d = nc.dram_tensor("xT", [NCHUNK, P, DO, 512], f16, kind="ExternalInput")
    w1_d = nc.dram_tensor("w1r", [FT, P, DO, P], f16, kind="ExternalInput")
    bc_d = nc.dram_tensor("bc", [P, FT + DMO], f32, kind="ExternalInput")
    w2_d = nc.dram_tensor("w2r", [DMO, 2, P, FT2, P], f16, kind="ExternalInput")
    yT_d = nc.dram_tensor("yT", [DMO, P, T], f32, kind="ExternalOutput")

    gelu = mybir.ActivationFunctionType.Gelu_apprx_tanh

    with tile.TileContext(nc) as tc:
        with (
            tc.tile_pool(name="xpool", bufs=1) as xpool,
            tc.tile_pool(name="hpool", bufs=1) as hpool,
            tc.tile_pool(name="wpool", bufs=2) as wpool,
            tc.tile_pool(name="cpool", bufs=1) as cpool,
            tc.tile_pool(name="ypool", bufs=2) as ypool,
            tc.tile_pool(name="psum_h", bufs=2, space="PSUM") as psum_h,
            tc.tile_pool(name="psum_y", bufs=2, space="PSUM") as psum_y,
        ):
            # scratch for PE warmup, prepared before anything else queues
            scratch = cpool.tile([P, 512], f16)
            nc.gpsimd.memset(scratch[:], 0.0)

            # Input DMAs, all on the sync HWDGE ring (FIFO, ~300GB/s) in
            # exact consumption order. Phase 1 is c-split (all f-tiles over
            # x chunk 0, then all over chunk 1), so only w1[0..k]+x chunk 0
            # are head-critical; chunk 1 has the whole c0 pass of slack.
            # All 32 w1 tiles stay resident (8MB) for the c1 pass.
            w1_tiles = {}
            for ft in range(8):
                w1_tiles[ft] = wpool.tile(
                    [P, DO, P], f16, tag="w1", bufs=FT, name="w1_sb"
                )
            # contiguous-per-partition destination: [p, c, do*512]
            xT_sb = xpool.tile([P, NCHUNK, DO * 512], f16)
            xc0 = xT_d.ap()[0].rearrange("p do t -> p (do t)")
            xc1 = xT_d.ap()[1].rearrange("p do t -> p (do t)")
            W = 512  # one do-tile worth of columns
            HX = DO * 512 // 2

            nc.sync.dma_start(w1_tiles[0][:], w1_d.ap()[0])
            bc_sb = cpool.tile([P, FT + DMO], f32)
            nc.sync.dma_start(bc_sb[:], bc_d.ap())
            nc.sync.dma_start(xT_sb[:, 0, : 2 * W], xc0[:, : 2 * W])
            nc.sync.dma_start(xT_sb[:, 0, 2 * W : 4 * W], xc0[:, 2 * W : 4 * W])
            nc.sync.dma_start(xT_sb[:, 0, 4 * W : 6 * W], xc0[:, 4 * W : 6 * W])
            nc.sync.dma_start(xT_sb[:, 0, 6 * W :], xc0[:, 6 * W :])
            nc.sync.dma_start(w1_tiles[1][:], w1_d.ap()[1])
            nc.sync.dma_start(w1_tiles[2][:], w1_d.ap()[2])
            nc.sync.dma_start(w1_tiles[3][:], w1_d.ap()[3])
            nc.sync.dma_start(w1_tiles[4][:], w1_d.ap()[4])
            nc.sync.dma_start(xT_sb[:, 1, :HX], xc1[:, :HX])
            nc.sync.dma_start(w1_tiles[5][:], w1_d.ap()[5])
            nc.sync.dma_start(w1_tiles[6][:], w1_d.ap()[6])
            nc.sync.dma_start(xT_sb[:, 1, HX:], xc1[:, HX:])
            nc.sync.dma_start(w1_tiles[7][:], w1_d.ap()[7])
            b1_sb = bc_sb[:, :FT]
            b2_sb = bc_sb[:, FT:]

            # PE warmup: dummy matmuls on scratch while the first DMAs land.
            # Keeps the HAM clock-gate fed until real work arrives.
            for i in range(N_WARMUP_MM):
                pw = psum_y.tile([P, 512], f32, tag="py", bufs=4, name="pwarm")
                nc.tensor.matmul(
                    pw[:], scratch[:, :P], scratch[:], start=True, stop=True
                )

            h_sb = hpool.tile([P, FT, T], f16)

            # ---- phase 1: h^T = gelu(w1^T x^T + b1), one 128-row f-tile at a time
            # per-(ft, chunk) 1-bank PSUM tiles; the first HEAD f-tiles run
            # chunk-0 first so the PE streams while x chunk 1 is on the wire
            def mm1_group(ph, w1_sb, c):
                mm = None
                for do in range(DO):
                    mm = nc.tensor.matmul(
                        ph[:],
                        w1_sb[:, do, :],
                        xT_sb[:, c, do * 512 : (do + 1) * 512],
                        start=(do == 0),
                        stop=(do == DO - 1),
                    )
                return mm

            def gelu_chunk(ph, ft, c):
                cs = slice(c * 512, (c + 1) * 512)
                return nc.scalar.activation(
                    h_sb[:, ft, cs], ph[:], gelu, bias=b1_sb[:, ft : ft + 1]
                )

            gelu_insts = {}
            for c in range(NCHUNK):
                for ft in range(FT):
                    if ft not in w1_tiles:
                        w1_tiles[ft] = wpool.tile(
                            [P, DO, P], f16, tag="w1", bufs=FT, name="w1_sb"
                        )
                        nc.sync.dma_start(w1_tiles[ft][:], w1_d.ap()[ft])
                    ph = psum_h.tile([P, 512], f32, tag="ph", bufs=4, name="ph")
                    mm1_group(ph, w1_tiles[ft], c)
                    gelu_insts[(ft, c)] = gelu_chunk(ph, ft, c)

            # ---- phase 2: y^T[dmo] = w2[:,dmo]^T h^T + b2[dmo], chunk-major:
            # each (dmo, c) group accumulates a 1-bank 512-col PSUM tile and
            # flushes it in 256-col pieces so the y DMA stream never bursts.
            # The very last group runs as two 256-col PSUM tiles so half its
            # flush overlaps the final matmuls, and its last DMA issues from
            # the (idle) scalar queue instead of the congested sync queue.
            FQ = FT // 4
            for dmo in range(DMO):
                w2_q = []
                for qq in range(4):
                    w2_sb = wpool.tile([P, FQ, P], f16, tag="w2", bufs=8, name="w2_sb")
                    dma = nc.sync.dma_start(
                        w2_sb[:],
                        w2_d.ap()[dmo, qq // 2, :, (qq % 2) * FQ : (qq % 2 + 1) * FQ],
                    )
                    if dmo <= 1:
                        # keep the first prefetches out of the head's
                        # w1/xT window (dmo>=2 is gated by buffer reuse)
                        add_dep_helper(
                            dma.ins,
                            gelu_insts[(8, 0)].ins,
                            sync=True,
                            reason="delay w2 prefetch past the kernel head",
                        )
                    w2_q.append(w2_sb)

                if dmo < DMO - 1:
                    for c in range(NCHUNK):
                        cs = slice(c * 512, (c + 1) * 512)
                        py = psum_y.tile([P, 512], f32, tag="py", bufs=4, name="py")
                        for fo in range(FT):
                            wt = w2_q[fo // FQ][:, fo % FQ, :]
                            nc.tensor.matmul(
                                py[:],
                                wt,
                                h_sb[:, fo, cs],
                                start=(fo == 0),
                                stop=(fo == FT - 1),
                            )
                        for cc in range(2):
                            ccs = slice(c * 512 + cc * 256, c * 512 + (cc + 1) * 256)
                            y_sb = ypool.tile([P, 256], f32, tag="y", bufs=4, name="y_sb")
                            nc.vector.tensor_scalar_add(
                                y_sb[:],
                                py[:, cc * 256 : (cc + 1) * 256],
                                b2_sb[:, dmo : dmo + 1],
                            )
                            # late y flushes ride the (idle) scalar ring so
                            # the sync ring's tail FIFO stays short
                            eng = nc.scalar if dmo >= 5 else nc.sync
                            eng.dma_start(yT_d.ap()[dmo, :, ccs], y_sb[:])
                else:
                    # last dm-tile, chunk-major in progressively smaller
                    # PSUM groups (512, 256, 128, 128 cols) so only a 128-col
                    # add + 64KB DMA trail the final matmul
                    for lo, w in ((0, 512), (512, 256), (768, 128), (896, 128)):
                        gs = slice(lo, lo + w)
                        py = psum_y.tile([P, w], f32, tag="py", bufs=4, name="pyl")
                        for fo in range(FT):
                            wt = w2_q[fo // FQ][:, fo % FQ, :]
                            nc.tensor.matmul(
                                py[:],
                                wt,
                                h_sb[:, fo, gs],
                                start=(fo == 0),
                                stop=(fo == FT - 1),
                            )
                        nf = 2 if w == 512 else 1
                        for cc in range(nf):
                            fw = w // nf
                            ccs = slice(lo + cc * fw, lo + (cc + 1) * fw)
                            y_sb = ypool.tile([P, fw], f32, tag="y", bufs=4, name="y_sb")
                            nc.vector.tensor_scalar_add(
                                y_sb[:],
                                py[:, cc * fw : (cc + 1) * fw],
                                b2_sb[:, dmo : dmo + 1],
                            )
                            nc.scalar.dma_start(yT_d.ap()[dmo, :, ccs], y_sb[:])

    nc.compile()
    return nc


def _get_nc():
    global _cached
    if _cached is None:
        _cached = _build()
    return _cached


def make_in_maps(x, w1, b1, w2, b2):
    x = np.asarray(x, dtype=np.float32)
    w1 = np.asarray(w1, dtype=np.float32)
    b1 = np.asarray(b1, dtype=np.float32)
    w2 = np.asarray(w2, dtype=np.float32)
    b2 = np.asarray(b2, dtype=np.float32)

    tokens = x.reshape(E, T, D)
    in_maps = []
    for e in range(E):
        xT = np.ascontiguousarray(
            tokens[e].reshape(NCHUNK, 512, DO, P).transpose(0, 3, 2, 1)
        ).astype(np.float16)  # [c, p, do, t']
        w1r = np.ascontiguousarray(
            w1[e].reshape(DO, P, FT, P).transpose(2, 1, 0, 3)
        ).astype(np.float16)  # [ft, p, do, j]
        bc = np.ascontiguousarray(
            np.concatenate([b1[e].reshape(FT, P).T, b2[e].reshape(DMO, P).T], axis=1)
        )  # [p, ft..dmo]
        w2r = np.ascontiguousarray(
            w2[e].reshape(2, FT2, P, DMO, P).transpose(3, 0, 2, 1, 4)
        ).astype(np.float16)  # [dmo, half, p, fo, j]
        in_maps.append({"xT": xT, "w1r": w1r, "bc": bc, "w2r": w2r})
    return in_maps


def gather_out(results, b2=None):
    out = np.empty((E, T, D), dtype=np.float32)
    for e in range(E):
        yT = results[e]["yT"]  # [dmo, p, t]
        out[e] = yT.transpose(2, 0, 1).reshape(T, D)
    return out.reshape(B, S, D)


def kernel(x, w1, b1, w2, b2):
    from concourse.bass_utils import run_bass_kernel_spmd

    nc = _get_nc()
    in_maps = make_in_maps(x, w1, b1, w2, b2)
    res = run_bass_kernel_spmd(nc, in_maps, core_ids=list(range(E)))
    return gather_out(res.results, b2)
